# revision 1
# baseline (speedup 1.0000x reference)
"""Trainium2 Bass kernel for nn_HG_60481729462790 (gnn_message_passing).

Computes, for 800k (rna, dis) edge pairs over a shared embedding table:
    norm  = ||emb_row||_2 + 0.1 per row; emb_n = emb / norm
    logit = emb_n[rna_i] @ We @ emb_n[20000 + dis_j]^T   per edge
    returns (concat(pos_logits, neg_logits), concat(ones, zeros))

Strategy (8 NeuronCores, data-parallel over edges):
  - Each core builds two DRAM tables (replicated work, cheap):
      rna_n[20096,128] = normalized emb rows 0..20095
      dwn  [5000,128]  = We @ normalized dis row (normalization folded in)
    so that  logit(e) = dot(rna_n[i_e], dwn[j_e]).
  - Edge stream is sharded 100k/core. Per 2048-edge block: two indirect-DMA
    gathers (128 partitions x 16 rows each; one 512B descriptor per row),
    a DVE elementwise multiply and a segmented reduce -> 2048 logits.
  - Outputs are concatenated on host; label vector is a constant.
"""

import numpy as np

import concourse.bass as bass
import concourse.bacc as bacc
import concourse.mybir as mybir
import concourse.tile as tile
from concourse.bass import IndirectOffsetOnAxis
from concourse.masks import make_identity
from concourse.tile_rust import add_dep_helper

P = 128            # partitions
H = 128            # hidden
N_EMB = 25000
N_RNA = 20000
N_DIS = 5000
E_TOT = 800000
N_CORES = 8
E_CORE = E_TOT // N_CORES          # 100000 edges per core
K = 16                             # gathered rows per partition per block
BLK = P * K                        # 2048 edges per block
N_FULL = E_CORE // BLK             # 48 full blocks
TAIL = E_CORE - N_FULL * BLK       # 1696
TAIL_P = TAIL // K                 # 106 partitions in the tail block
GROUP = 8                          # blocks per output flush (16384 edges)

N_RNA_PAD = ((N_RNA + P - 1) // P) * P   # 20096 (rows 20000..20095 are
                                         # normalized dis rows; never indexed)

F32 = mybir.dt.float32
I32 = mybir.dt.int32

TABLE_DT = F32     # dtype of the gather tables (flip to bfloat16 to A/B)


ST = 8             # row-tiles per phase-A supertile (1024 rows per iteration)


def _norm_scale_wide(nc, pool, x_ap, p_used, tw):
    """Scale tile [P, tw] with 1/(||row||+0.1) for rows of x_ap [p_used, tw, H].

    Sum-of-squares runs on DVE so the ACT engine only ever executes Sqrt —
    mixing activation functions reloads the 1.3us activation table per op.
    """
    sq = pool.tile([P, ST, H], F32, tag="sq")
    sq2 = sq[:p_used, :tw, :].rearrange("p t h -> p (t h)")
    x2 = x_ap.rearrange("p t h -> p (t h)")
    nc.vector.tensor_tensor(out=sq2, in0=x2, in1=x2, op=mybir.AluOpType.mult)
    ss = pool.tile([P, ST], F32, tag="ss")
    nc.vector.reduce_sum(out=ss[:p_used, :tw], in_=sq[:p_used, :tw, :],
                         axis=mybir.AxisListType.X)
    nc.scalar.activation(
        out=ss[:p_used, :tw], in_=ss[:p_used, :tw],
        func=mybir.ActivationFunctionType.Sqrt,
    )
    nc.vector.tensor_scalar_add(ss[:p_used, :tw], ss[:p_used, :tw], 0.1)
    rec = pool.tile([P, ST], F32, tag="rec")
    nc.vector.reciprocal(rec[:p_used, :tw], ss[:p_used, :tw])
    return rec


def geom(k):
    blk = P * k
    n_full = E_CORE // blk
    tail = E_CORE - n_full * blk
    group = 128 // k          # blocks per score tile so GROUP*k == 128
    n_blocks = n_full + (1 if tail else 0)
    return blk, n_full, tail, group, n_blocks


def build_program(table_dt=TABLE_DT, phase_a=True, phase_b=True,
                  gathers=2, compute=True, flush=True, repeat_b=1,
                  k=K, single_packet=False):
    blk, n_full, tail, group, _nb = geom(k)
    nc = bacc.Bacc()

    emb = nc.dram_tensor("emb", [N_EMB, H], F32, kind="ExternalInput")
    we = nc.dram_tensor("We", [H, H], F32, kind="ExternalInput")
    # edge indices, host-prepared in dma_gather's wrapped-int16 layout:
    # plane column block b holds block b's 2048 indices with element i at
    # [i % 16, i // 16] (tail block padded with -1 = "skip")
    n_blocks = _nb
    ipw = blk // 16                       # index-plane columns per block
    ridx = nc.dram_tensor("rna_idx16", [16, n_blocks * ipw], mybir.dt.int16,
                          kind="ExternalInput")
    didx = nc.dram_tensor("dis_idx16", [16, n_blocks * ipw], mybir.dt.int16,
                          kind="ExternalInput")
    out = nc.dram_tensor("logits", [E_CORE], F32, kind="ExternalOutput")

    rna_n = nc.dram_tensor("rna_n", [N_RNA_PAD, H], table_dt, kind="Internal")
    dwn = nc.dram_tensor("dwn", [N_DIS, H], table_dt, kind="Internal")

    table_writes = []

    with tile.TileContext(nc) as tc:
        with (
            tc.tile_pool(name="const", bufs=1) as const_pool,
            tc.tile_pool(name="pa", bufs=3) as pa_pool,
            tc.tile_pool(name="pa_small", bufs=4) as pa_small,
            tc.tile_pool(name="pa_psum", bufs=2, space="PSUM") as pa_psum,
            tc.tile_pool(name="pb_idx", bufs=4) as pb_idx,
            tc.tile_pool(name="pb_gat", bufs=3) as pb_gat,
            tc.tile_pool(name="pb_sc", bufs=2) as pb_sc,
        ):
            # --- Phase A0: We^T in SBUF ------------------------------------
            # PE (Matmult) instructions tolerate only ONE sync-wait in walrus
            # codegen, so every PE input must be produced by the same engine
            # (DVE): route identity and We through DVE copies.
            ident0 = const_pool.tile([P, P], F32)
            make_identity(nc, ident0[:])
            ident = const_pool.tile([P, P], F32)
            nc.vector.tensor_copy(out=ident[:], in_=ident0[:])
            we_s0 = const_pool.tile([P, H], F32)
            nc.sync.dma_start(out=we_s0[:], in_=we[:, :])
            we_s = const_pool.tile([P, H], F32)
            nc.vector.tensor_copy(out=we_s[:], in_=we_s0[:])
            wet_ps = pa_psum.tile([P, P], F32, tag="wet")
            nc.tensor.transpose(out=wet_ps[:], in_=we_s[:], identity=ident[:])
            wet = const_pool.tile([P, H], F32)   # wet[h, ho] = We[ho, h]
            nc.vector.tensor_copy(out=wet[:], in_=wet_ps[:])

            # --- Phase A1: normalized rna table (rows 0..20095) ------------
            # Supertiles of ST row-tiles: wide DVE ops amortize the fixed
            # per-instruction overhead that dominated the per-tile version.
            n_st = (N_RNA_PAD // P + ST - 1) // ST if phase_a else 0
            for st in range(n_st):
                r0 = st * ST * P
                tw = min(ST, (N_RNA_PAD - r0) // P)
                x = pa_pool.tile([P, ST, H], F32, tag="ax")
                nc.sync.dma_start(
                    out=x[:, :tw, :],
                    in_=emb[r0:r0 + tw * P, :].rearrange("(t p) h -> p t h", p=P))
                rec = _norm_scale_wide(nc, pa_small, x[:, :tw, :], P, tw)
                y = pa_pool.tile([P, ST, H], table_dt, tag="ay")
                nc.vector.tensor_tensor(
                    out=y[:, :tw, :], in0=x[:, :tw, :],
                    in1=rec[:, :tw].to_broadcast([P, tw, H]),
                    op=mybir.AluOpType.mult)
                w = nc.sync.dma_start(
                    out=rna_n[r0:r0 + tw * P, :].rearrange("(t p) h -> p t h", p=P),
                    in_=y[:, :tw, :])
                table_writes.append(w)

            # --- Phase A2: dwn table (We @ normalized dis rows) ------------
            # norm on supertiles; transpose+matmul per 128-row subtile
            N_DIS_FULL = (N_DIS // P) * P               # 4992
            n_dst = (N_DIS_FULL // P + ST - 1) // ST if phase_a else 0
            for st in range(n_dst):
                d0 = st * ST * P
                tw = min(ST, (N_DIS_FULL - d0) // P)
                x = pa_pool.tile([P, ST, H], F32, tag="dx")
                nc.sync.dma_start(
                    out=x[:, :tw, :],
                    in_=emb[N_RNA + d0:N_RNA + d0 + tw * P, :].rearrange(
                        "(t p) h -> p t h", p=P))
                rec = _norm_scale_wide(nc, pa_small, x[:, :tw, :], P, tw)
                dn = pa_pool.tile([P, ST, H], F32, tag="dn")
                nc.vector.tensor_tensor(
                    out=dn[:, :tw, :], in0=x[:, :tw, :],
                    in1=rec[:, :tw].to_broadcast([P, tw, H]),
                    op=mybir.AluOpType.mult)
                for t in range(tw):
                    dnt_ps = pa_psum.tile([P, P], F32, tag="dnt")
                    nc.tensor.transpose(out=dnt_ps[:], in_=dn[:, t, :],
                                        identity=ident[:])
                    dnt = pa_pool.tile([P, P], F32, tag="dnts")
                    nc.vector.tensor_copy(out=dnt[:], in_=dnt_ps[:])
                    # out[d, ho] = sum_h dn[d,h] * We[ho,h]
                    mm_ps = pa_psum.tile([P, H], F32, tag="mm")
                    nc.tensor.matmul(out=mm_ps[:], lhsT=dnt[:],
                                     rhs=wet[:], start=True, stop=True)
                    z = pa_pool.tile([P, H], table_dt, tag="az")
                    nc.vector.tensor_copy(out=z[:], in_=mm_ps[:])
                    w = nc.sync.dma_start(
                        out=dwn[d0 + t * P:d0 + (t + 1) * P, :], in_=z[:])
                    table_writes.append(w)

            # ragged dis tail (rows 4992..4999)
            for _ in range(1 if phase_a else 0):
                rows = N_DIS - N_DIS_FULL                # 8
                d0 = N_DIS_FULL
                x = pa_pool.tile([P, ST, H], F32, tag="dx")
                nc.sync.dma_start(
                    out=x[:rows, 0, :],
                    in_=emb[N_RNA + d0:N_RNA + d0 + rows, :])
                rec = _norm_scale_wide(nc, pa_small, x[:rows, :1, :], rows, 1)
                dn = pa_pool.tile([P, ST, H], F32, tag="dn")
                nc.vector.tensor_tensor(
                    out=dn[:rows, :1, :], in0=x[:rows, :1, :],
                    in1=rec[:rows, :1].to_broadcast([rows, 1, H]),
                    op=mybir.AluOpType.mult)
                dnt_ps = pa_psum.tile([P, P], F32, tag="dnt")
                nc.tensor.transpose(out=dnt_ps[:, :rows], in_=dn[:rows, 0, :],
                                    identity=ident[:rows, :rows])
                dnt = pa_pool.tile([P, P], F32, tag="dnts")
                nc.vector.tensor_copy(out=dnt[:, :rows], in_=dnt_ps[:, :rows])
                mm_ps = pa_psum.tile([P, H], F32, tag="mm")
                nc.tensor.matmul(out=mm_ps[:rows, :], lhsT=dnt[:, :rows],
                                 rhs=wet[:], start=True, stop=True)
                z = pa_pool.tile([P, H], table_dt, tag="az")
                nc.vector.tensor_copy(out=z[:rows], in_=mm_ps[:rows, :])
                w = nc.sync.dma_start(out=dwn[d0:d0 + rows, :], in_=z[:rows])
                table_writes.append(w)

            # Fence: every gather must observe the completed tables.
            fence = nc.gpsimd.nop(nofuse=True, hint="table_fence")
            for w in table_writes:
                add_dep_helper(fence.ins, w.ins, reason="fence waits on table writes")

            # --- Phase B: gather + dot per 2048-edge block -----------------
            # dma_gather places row i at dest[p=i%128, slot=i//128], so edge
            # q (block-local) = s*128 + p lands at score[p, s].  A group of 8
            # blocks gives score_grp[128, 128] with col c = b_loc*16 + s; its
            # PE transpose has row c = 128 contiguous output edges.
            # Load both wrapped index planes once, replicated 8x down the
            # partition axis (one 16-partition copy per pair of Q7 cores).
            ridx_s = const_pool.tile([P, n_blocks * ipw], mybir.dt.int16)
            didx_s = const_pool.tile([P, n_blocks * ipw], mybir.dt.int16)
            for c in range(8):
                nc.sync.dma_start(out=ridx_s[16 * c:16 * (c + 1), :], in_=ridx[:, :])
                nc.sync.dma_start(out=didx_s[16 * c:16 * (c + 1), :], in_=didx[:, :])

            score = None
            for _rep_b in range((n_blocks if phase_b else 0) * repeat_b):
                b = _rep_b % max(n_blocks, 1)
                full = b < n_full
                n_valid = blk if full else tail

                g = b % group
                if g == 0:
                    score = pb_sc.tile([P, group * k], F32, tag="score")
                    if n_blocks - b < group:
                        # partial final group: zero unused columns so the
                        # full-tile transpose reads defined data
                        nc.vector.memset(score[:, :], 0.0)

                r = pb_gat.tile([P, k, H], table_dt, tag="r")
                d = pb_gat.tile([P, k, H], table_dt, tag="d")
                if not full:
                    # tail: gather skips the -1-padded rows; zero the tiles so
                    # the full-tile multiply/reduce reads defined data
                    nc.vector.memset(r[:, :, :], 0.0)
                    nc.vector.memset(d[:, :, :], 0.0)
                g1 = nc.gpsimd.dma_gather(
                    r[:, :, :], rna_n[:, :], ridx_s[:, b * ipw:(b + 1) * ipw],
                    num_idxs=blk, num_idxs_reg=n_valid,
                    elem_size=H, elem_step=H, single_packet=single_packet)
                add_dep_helper(g1.ins, fence.ins, reason="gather after tables")
                if gathers >= 2:
                    g2 = nc.gpsimd.dma_gather(
                        d[:, :, :], dwn[:, :], didx_s[:, b * ipw:(b + 1) * ipw],
                        num_idxs=blk, num_idxs_reg=n_valid,
                        elem_size=H, elem_step=H, single_packet=single_packet)
                    add_dep_helper(g2.ins, fence.ins, reason="gather after tables")
                elif compute:
                    nc.vector.memset(d[:, :, :], 0.5)

                if compute:
                    r2 = r[:, :, :].rearrange("p s h -> p (s h)")
                    d2 = d[:, :, :].rearrange("p s h -> p (s h)")
                    nc.vector.tensor_mul(r2, r2, d2)
                    nc.vector.reduce_sum(
                        out=score[:, g * k:(g + 1) * k], in_=r[:, :, :],
                        axis=mybir.AxisListType.X)

                # flush finished group via PE transpose -> contiguous DMA
                last_in_group = (g == group - 1) or (b == n_blocks - 1)
                if last_in_group and compute and flush:
                    e_g0 = (b - g) * blk
                    st_ps = pa_psum.tile([P, P], F32, tag="st")
                    nc.tensor.transpose(out=st_ps[:], in_=score[:, :],
                                        identity=ident[:])
                    st = pb_sc.tile([P, P], F32, tag="st_s")
                    nc.vector.tensor_copy(out=st[:], in_=st_ps[:])
                    n_out = (g * blk) + n_valid          # edges in this group
                    rows, rem = divmod(n_out, P)
                    if rows:
                        nc.sync.dma_start(
                            out=out[e_g0:e_g0 + rows * P].rearrange(
                                "(c p) -> c p", p=P),
                            in_=st[:rows, :])
                    if rem:
                        nc.sync.dma_start(
                            out=out[e_g0 + rows * P:e_g0 + n_out].rearrange(
                                "(o e) -> o e", o=1),
                            in_=st[rows:rows + 1, :rem])

    # Bacc pipeline: splits multi-waits into event semaphores (walrus allows
    # only one sync-wait per instruction), register alloc, DCE, etc.
    nc.compile()
    return nc


_PROGRAM_CACHE = {}


def _get_program(table_dt=TABLE_DT, k=K, single_packet=False):
    key = (str(table_dt), k, single_packet)
    if key not in _PROGRAM_CACHE:
        _PROGRAM_CACHE[key] = build_program(
            table_dt, k=k, single_packet=single_packet)
    return _PROGRAM_CACHE[key]


def wrap_indices(idx, k=K):
    """[E_CORE] int -> dma_gather wrapped plane [16, n_blocks*k] int16.

    Block b's blk indices (tail padded with -1) occupy plane columns
    [b*k, (b+1)*k) with element i at [i % 16, i // 16].
    """
    blk, n_full, tail, group, n_blocks = geom(k)
    padded = np.full(n_blocks * blk, -1, dtype=np.int16)
    padded[:len(idx)] = idx.astype(np.int16)
    blocks = padded.reshape(n_blocks, blk // 16, 16).transpose(0, 2, 1)
    return np.ascontiguousarray(
        blocks.transpose(1, 0, 2).reshape(16, n_blocks * (blk // 16)))


def _make_in_maps(emb, We, rna_all, dis_all, k=K):
    in_maps = []
    for c in range(N_CORES):
        sl = slice(c * E_CORE, (c + 1) * E_CORE)
        in_maps.append({
            "emb": np.ascontiguousarray(emb, dtype=np.float32),
            "We": np.ascontiguousarray(We, dtype=np.float32),
            "rna_idx16": wrap_indices(np.asarray(rna_all[sl]), k),
            "dis_idx16": wrap_indices(np.asarray(dis_all[sl]), k),
        })
    return in_maps


def kernel_run(emb, We, pos_rna, pos_dis, neg_rna, neg_dis, rna_num,
               trace=False, table_dt=TABLE_DT, k=K, single_packet=False):
    """Returns ((logits, label), exec_time_ns_or_None)."""
    from concourse.bass_utils import run_bass_kernel_spmd

    emb = np.asarray(emb)
    We = np.asarray(We)
    rna_all = np.concatenate([np.asarray(pos_rna), np.asarray(neg_rna)])
    dis_all = np.concatenate([np.asarray(pos_dis), np.asarray(neg_dis)])
    assert emb.shape == (N_EMB, H) and We.shape == (H, H)
    assert rna_all.shape == (E_TOT,) and dis_all.shape == (E_TOT,)

    nc = _get_program(table_dt, k, single_packet)
    in_maps = _make_in_maps(emb, We, rna_all, dis_all, k)
    res = run_bass_kernel_spmd(
        nc, in_maps, core_ids=list(range(N_CORES)), trace=trace)

    logits = np.concatenate([res.results[c]["logits"] for c in range(N_CORES)])
    n_pos = np.asarray(pos_rna).shape[0]
    n_neg = np.asarray(neg_rna).shape[0]
    label = np.concatenate([np.ones(n_pos, np.float32),
                            np.zeros(n_neg, np.float32)])
    return (logits.astype(np.float32), label), res.exec_time_ns


def kernel(**inputs):
    (logits, label), _ = kernel_run_v2(**inputs)
    return (logits, label)


# ============================ V2: j-binned PE-select =========================
# dis side descriptor-free: bin edges by j//128 (40 bins, capacity C), keep
# dwnT = (We @ dis_n^T) [h, 5120] resident in SBUF; per bin one transpose-mode
# rna gather (bf16, ~n_b descriptors) gives rT [h, C]; per 1024-edge chunk,
# PE computes S^T = dwnT_B^T-contract = [j'=128, e] and a host-streamed
# one-hot plane OH [j', e] selects each edge's j' column:
#   logit[e] = sum_j' OH[j',e] * S^T[j',e]   (DVE mult + PE ones-reduce)
# Q7 descriptor work halves vs the two-gather baseline.

N_BINS = 40
CAP = 3072                 # slots per bin (mean 2500, sigma ~50)
N_CHUNK = CAP // 1024      # 3 chunks per bin
TOT_SLOTS = N_BINS * CAP   # 122880
TOT_CHUNKS = N_BINS * N_CHUNK


def build_program_v2():
    nc = bacc.Bacc()
    BF16 = mybir.dt.bfloat16

    emb = nc.dram_tensor("emb", [N_EMB, H], F32, kind="ExternalInput")
    we = nc.dram_tensor("We", [H, H], F32, kind="ExternalInput")
    ridx = nc.dram_tensor("rna_idx16", [16, TOT_SLOTS // 16], mybir.dt.int16,
                          kind="ExternalInput")
    oh = nc.dram_tensor("oh", [TOT_CHUNKS, P, 1024], BF16, kind="ExternalInput")
    counts = nc.dram_tensor("counts", [1, N_BINS], I32, kind="ExternalInput")
    out = nc.dram_tensor("logits", [TOT_SLOTS], F32, kind="ExternalOutput")

    rna_n = nc.dram_tensor("rna_n", [N_RNA_PAD, H], BF16, kind="Internal")

    table_writes = []

    with tile.TileContext(nc) as tc:
        with tc.tile_pool(name="const", bufs=1) as const_pool:
            # --- constants ------------------------------------------------
            ident0 = const_pool.tile([P, P], F32)
            make_identity(nc, ident0[:])
            ident = const_pool.tile([P, P], F32)
            nc.vector.tensor_copy(out=ident[:], in_=ident0[:])
            we_s0 = const_pool.tile([P, H], F32)
            nc.sync.dma_start(out=we_s0[:], in_=we[:, :])
            we_s = const_pool.tile([P, H], F32)
            nc.vector.tensor_copy(out=we_s[:], in_=we_s0[:])
            ones_b = const_pool.tile([P, 1], BF16)
            nc.vector.memset(ones_b[:, :], 1.0)
            dwnT = const_pool.tile([P, N_BINS * P], BF16)   # [h, j'] table
            nc.vector.memset(dwnT[:, :], 0.0)
            cnt_sb = const_pool.tile([1, N_BINS], I32)
            nc.sync.dma_start(out=cnt_sb[:, :], in_=counts[:, :])
            ridx_s = const_pool.tile([P, TOT_SLOTS // 16], mybir.dt.int16)
            for c in range(8):
                nc.sync.dma_start(out=ridx_s[16 * c:16 * (c + 1), :],
                                  in_=ridx[:, :])

            # --- phase A (own pools; PSUM freed before phase B) -----------
            with (
                tc.tile_pool(name="pa", bufs=3) as pa_pool,
                tc.tile_pool(name="pa_small", bufs=4) as pa_small,
                tc.tile_pool(name="pa_psum", bufs=2, space="PSUM") as pa_psum,
            ):
                wet_ps = pa_psum.tile([P, P], F32, tag="wet")
                nc.tensor.transpose(out=wet_ps[:], in_=we_s[:], identity=ident[:])
                wet = const_pool.tile([P, H], F32)   # wet[h, ho] = We[ho, h]
                nc.vector.tensor_copy(out=wet[:], in_=wet_ps[:])

                # A1: normalized rna table in bf16 (rows 0..20095)
                n_st = (N_RNA_PAD // P + ST - 1) // ST
                for st in range(n_st):
                    r0 = st * ST * P
                    tw = min(ST, (N_RNA_PAD - r0) // P)
                    x = pa_pool.tile([P, ST, H], F32, tag="ax")
                    nc.sync.dma_start(
                        out=x[:, :tw, :],
                        in_=emb[r0:r0 + tw * P, :].rearrange(
                            "(t p) h -> p t h", p=P))
                    rec = _norm_scale_wide(nc, pa_small, x[:, :tw, :], P, tw)
                    y = pa_pool.tile([P, ST, H], BF16, tag="ay")
                    nc.vector.tensor_tensor(
                        out=y[:, :tw, :], in0=x[:, :tw, :],
                        in1=rec[:, :tw].to_broadcast([P, tw, H]),
                        op=mybir.AluOpType.mult)
                    w = nc.sync.dma_start(
                        out=rna_n[r0:r0 + tw * P, :].rearrange(
                            "(t p) h -> p t h", p=P),
                        in_=y[:, :tw, :])
                    table_writes.append(w)

                # A2: dwnT = We @ dis_n^T into SBUF [h, 5120]
                # per 128-row dis tile: norm -> PE transpose -> dnt [h, d]
                # batches of 4 dnt tiles -> one [128,512] matmul vs wet
                dntbuf = None
                for t in range(N_BINS):
                    d0 = t * P
                    rows = min(P, N_DIS - d0)
                    x = pa_pool.tile([P, ST, H], F32, tag="dx")
                    if rows < P:
                        nc.vector.memset(x[:, 0, :], 0.0)
                    nc.sync.dma_start(
                        out=x[:rows, 0, :],
                        in_=emb[N_RNA + d0:N_RNA + d0 + rows, :])
                    rec = _norm_scale_wide(nc, pa_small, x[:, :1, :], P, 1)
                    dn = pa_pool.tile([P, ST, H], F32, tag="dn")
                    nc.vector.tensor_tensor(
                        out=dn[:, :1, :], in0=x[:, :1, :],
                        in1=rec[:, :1].to_broadcast([P, 1, H]),
                        op=mybir.AluOpType.mult)
                    if t % 4 == 0:
                        dntbuf = pa_pool.tile([P, 4 * P], F32, tag="dnt4")
                    dnt_ps = pa_psum.tile([P, P], F32, tag="dnt")
                    nc.tensor.transpose(out=dnt_ps[:], in_=dn[:, 0, :],
                                        identity=ident[:])
                    nc.vector.tensor_copy(
                        out=dntbuf[:, (t % 4) * P:(t % 4 + 1) * P],
                        in_=dnt_ps[:])
                    if t % 4 == 3:
                        mm_ps = pa_psum.tile([P, 4 * P], F32, tag="mm")
                        nc.tensor.matmul(out=mm_ps[:], lhsT=wet[:],
                                         rhs=dntbuf[:, :], start=True,
                                         stop=True)
                        nc.vector.tensor_copy(
                            out=dwnT[:, (t - 3) * P:(t + 1) * P],
                            in_=mm_ps[:])

            fence = nc.gpsimd.nop(nofuse=True, hint="table_fence")
            for w in table_writes:
                add_dep_helper(fence.ins, w.ins, reason="fence on rna_n")

            # --- phase B --------------------------------------------------
            with (
                tc.tile_pool(name="gat", bufs=3) as gat_pool,
                tc.tile_pool(name="ohp", bufs=3) as oh_pool,
                tc.tile_pool(name="pp", bufs=3) as p_pool,
                tc.tile_pool(name="sc", bufs=3) as sc_pool,
                tc.tile_pool(name="ps_st", bufs=2, space="PSUM") as ps_st,
                tc.tile_pool(name="ps_sc", bufs=2, space="PSUM") as ps_sc,
            ):
                ipw = CAP // 16
                for _w in range(3):      # pre-poison-clear the 3 rotating bufs
                    rt0 = gat_pool.tile([P, 1, CAP], BF16, tag="rt")
                    nc.vector.memset(rt0[:, :, :], 0.0)
                for b in range(N_BINS):
                    rt = gat_pool.tile([P, 1, CAP], BF16, tag="rt")
                    g = nc.gpsimd.dma_gather(
                        rt[:, :, :], rna_n[:, :],
                        ridx_s[:, b * ipw:(b + 1) * ipw],
                        num_idxs=CAP, num_idxs_reg=CAP,
                        elem_size=H, transpose=True, single_packet=False)
                    add_dep_helper(g.ins, fence.ins, reason="gather after A")
                    for cch in range(N_CHUNK):
                        e0 = cch * 1024
                        st_ps = ps_st.tile([P, 1024], F32, tag="st")
                        for hh in range(2):
                            nc.tensor.matmul(
                                out=st_ps[:, hh * 512:(hh + 1) * 512],
                                lhsT=dwnT[:, b * P:(b + 1) * P],
                                rhs=rt[:, 0, e0 + hh * 512:e0 + (hh + 1) * 512],
                                start=True, stop=True)
                        oh_t = oh_pool.tile([P, 1024], BF16, tag="oh")
                        nc.sync.dma_start(
                            out=oh_t[:, :],
                            in_=oh[b * N_CHUNK + cch, :, :])
                        p_t = p_pool.tile([P, 1024], BF16, tag="p")
                        nc.vector.tensor_tensor(
                            out=p_t[:, :], in0=oh_t[:, :], in1=st_ps[:, :],
                            op=mybir.AluOpType.mult)
                        sc_ps = ps_sc.tile([1, 1024], F32, tag="sc")
                        for hh in range(2):
                            nc.tensor.matmul(
                                out=sc_ps[0:1, hh * 512:(hh + 1) * 512],
                                lhsT=ones_b[:, :],
                                rhs=p_t[:, hh * 512:(hh + 1) * 512],
                                start=True, stop=True)
                        sc = sc_pool.tile([1, 1024], F32, tag="scs")
                        nc.vector.tensor_copy(out=sc[0:1, :], in_=sc_ps[0:1, :])
                        nc.sync.dma_start(
                            out=out[b * CAP + e0:b * CAP + e0 + 1024]
                                .rearrange("(o e) -> o e", o=1),
                            in_=sc[0:1, :])

    nc.compile()
    return nc


def _prep_v2(rna_core, dis_core):
    """Bin one core's edges by j//128. Returns (idx plane, oh, counts, orig)."""
    import ml_dtypes
    jb = dis_core // P
    order = np.argsort(jb, kind="stable")
    sorted_jb = jb[order]
    starts = np.searchsorted(sorted_jb, np.arange(N_BINS), side="left")
    ends = np.searchsorted(sorted_jb, np.arange(N_BINS), side="right")
    n_b = ends - starts
    assert (n_b <= CAP).all(), n_b.max()
    slots_idx = np.zeros(TOT_SLOTS, np.int16)
    o_slot = np.full(TOT_SLOTS, -1, np.int16)
    orig = np.full(TOT_SLOTS, -1, np.int64)
    for b in range(N_BINS):
        k = n_b[b]
        sel = order[starts[b]:ends[b]]
        slots_idx[b * CAP:b * CAP + k] = rna_core[sel].astype(np.int16)
        o_slot[b * CAP:b * CAP + k] = (dis_core[sel] % P).astype(np.int16)
        orig[b * CAP:b * CAP + k] = sel
    plane = np.ascontiguousarray(
        slots_idx.reshape(TOT_SLOTS // 16, 16).T)
    o_mat = o_slot.reshape(TOT_CHUNKS, 1024)
    oh = (np.arange(P, dtype=np.int16)[None, :, None] == o_mat[:, None, :]
          ).astype(ml_dtypes.bfloat16)
    counts = np.full(N_BINS, CAP, np.int32)
    return plane, oh, counts.reshape(1, N_BINS), orig


def kernel_run_v2(emb, We, pos_rna, pos_dis, neg_rna, neg_dis, rna_num,
                  trace=False):
    from concourse.bass_utils import run_bass_kernel_spmd

    emb = np.ascontiguousarray(np.asarray(emb), dtype=np.float32)
    We = np.ascontiguousarray(np.asarray(We), dtype=np.float32)
    rna_all = np.concatenate([np.asarray(pos_rna), np.asarray(neg_rna)])
    dis_all = np.concatenate([np.asarray(pos_dis), np.asarray(neg_dis)])

    key = "v2"
    if key not in _PROGRAM_CACHE:
        _PROGRAM_CACHE[key] = build_program_v2()
    nc = _PROGRAM_CACHE[key]

    in_maps, origs = [], []
    for c in range(N_CORES):
        sl = slice(c * E_CORE, (c + 1) * E_CORE)
        plane, oh, cnts, orig = _prep_v2(rna_all[sl], dis_all[sl])
        in_maps.append({"emb": emb, "We": We, "rna_idx16": plane,
                        "oh": oh, "counts": cnts})
        origs.append(orig)

    res = run_bass_kernel_spmd(
        nc, in_maps, core_ids=list(range(N_CORES)), trace=trace)

    logits = np.empty(E_TOT, np.float32)
    for c in range(N_CORES):
        pad = res.results[c]["logits"]
        orig = origs[c]
        m = orig >= 0
        logits[c * E_CORE + orig[m]] = pad[m]
    n_pos = np.asarray(pos_rna).shape[0]
    label = np.concatenate([np.ones(n_pos, np.float32),
                            np.zeros(E_TOT - n_pos, np.float32)])
    return (logits, label), res.exec_time_ns

